# revision 1
# baseline (speedup 1.0000x reference)
"""StyleGAN2-style modulated 3x3 conv (B=8, Ci=Co=512, H=W=32) on 8 TRN2 NeuronCores.

Sharding: data-parallel over batch, one sample per core (embarrassingly
parallel, no collectives). Per core the conv is computed as 9 shifted
matmuls over a zero-padded 34x34 image held in SBUF, contracting over
Ci in 128-chunks with fp32 accumulation in PSUM; compute dtype bf16.

Math (per sample b, with s = (Ci*K*K)**-0.5 folded out of both the conv
and the demod norm so the weights can be used unscaled):
  conv = conv2d(x * y_s, weight)                     # raw, no s
  xs2[o] = sum_i y_s[i]^2 * w2[i,o],  w2 = sum_k weight[o,i,k]^2
  out = conv / sqrt(xs2 + 1e-8 * Ci * K * K) + bias

Host-side prep is layout only plus the input-independent w2 fold
(a pure weight transform, shipped as a 10th "k-slot" of the weight
tensor); all input-dependent math runs on device.
"""

import numpy as np
import ml_dtypes

import concourse.mybir as mybir
from concourse import bacc
from concourse.tile import TileContext
from concourse.bass_utils import run_bass_kernel_spmd

B = 8
CI = 512
CO = 512
H = W = 32
KK = 9  # 3x3
NCI = CI // 128
NCO = CO // 128
HWPAD = 34
EPS_EFF = 1e-8 * CI * KK  # demod eps compensated for unscaled weights

F32 = mybir.dt.float32
BF16 = mybir.dt.bfloat16
AF = mybir.ActivationFunctionType


def build_nc():
    nc = bacc.Bacc("TRN2", target_bir_lowering=False, debug=False)

    x_ext = nc.declare_dram_parameter("x", [NCI, 128, H, W], BF16, isOutput=False)
    # cols 0..3 = y_s per ci-tile, cols 4..7 = bias per co-tile
    yb_ext = nc.declare_dram_parameter("yb", [128, 2 * NCI], F32, isOutput=False)
    # [jci, jco, ci_p, k(9)+w2(1), co_c] bf16
    wt_ext = nc.declare_dram_parameter(
        "wt", [NCI, NCO, 128, KK + 1, 128], BF16, isOutput=False
    )
    out_ext = nc.declare_dram_parameter("out", [NCO, 128, H * W], F32, isOutput=True)

    with TileContext(nc) as tc:
        with (
            tc.tile_pool(name="singles", bufs=1) as singles,
            tc.tile_pool(name="wts", bufs=1) as wts,
            tc.tile_pool(name="pads", bufs=1) as pads,
            tc.tile_pool(name="xin", bufs=4) as xin,
            tc.tile_pool(name="outs", bufs=3) as outs,
            tc.tile_pool(name="cps", bufs=6, space="PSUM") as cps,
            tc.tile_pool(name="dps", bufs=1, space="PSUM") as dps,
            tc.tile_pool(name="wps", bufs=1, space="PSUM") as wps,
        ):
            # ---- input DMAs ----
            # x on SP queues first (needed first); weights issued from the
            # otherwise-idle ACT engine at co-quarter granularity so the
            # first conv groups don't wait on whole-weight transfers.
            # DMA-ring bandwidth shares round-robin per ACTIVE transfer, so
            # the first-needed tensors (x0, yb, first weight quarter) only
            # land early if little else is in flight. Sync issues just x0+yb;
            # x1..x3 are issued from DVE after each previous x is consumed;
            # the weight stream is throttled on ACT with dummy 1-elem reads
            # (<=2 weight transfers in flight) until x has landed.
            xt_sb = [
                xin.tile([128, H, W], BF16, tag=f"x{j}", name=f"xt{j}")
                for j in range(NCI)
            ]
            yb_sb = singles.tile([128, 2 * NCI], F32)
            nc.sync.dma_start(out=xt_sb[0], in_=x_ext[0])
            nc.sync.dma_start(out=yb_sb, in_=yb_ext[:, :])

            wt_sb = [[None] * NCO for _ in range(NCI)]
            wscr = singles.tile([1, 1], F32)

            def wdma(j, q):
                w = wts.tile([128, KK + 1, 128], BF16, tag=f"wt{j}_{q}")
                nc.scalar.dma_start(out=w, in_=wt_ext[j, q])
                wt_sb[j][q] = w

            def wthrottle(j, q):
                nc.scalar.activation(
                    out=wscr, in_=wt_sb[j][q][0:1, 0, 0:1], func=AF.Copy
                )

            wdma(0, 0)
            wdma(1, 0)
            for q in range(2):
                for j in range(NCI):
                    nxt = (q * NCI + j) + 2  # keep two transfers ahead
                    if nxt < 2 * NCI:
                        wdma(nxt % NCI, nxt // NCI)
                    wthrottle(j, q)
            for q in range(2, NCO):
                for j in range(NCI):
                    wdma(j, q)

            def wt_slice(j, jo, k):
                return wt_sb[j][jo][:, k, :]

            # ---- PE warm-up: throwaway matmuls on memset data so the
            # HAM clock gate starts releasing before the real stream ----
            warm_lhs = singles.tile([128, 1], BF16)
            nc.vector.memset(warm_lhs, 1.0)
            warm_rhs = singles.tile([128, 512], BF16)
            nc.vector.memset(warm_rhs, 0.5)
            warm_ps = wps.tile([1, 512], F32)
            N_WARM = 5
            for i in range(N_WARM):
                nc.tensor.matmul(
                    out=warm_ps,
                    lhsT=warm_lhs,
                    rhs=warm_rhs,
                    start=(i == 0),
                    stop=(i == N_WARM - 1),
                )

            eps_sb = singles.tile([128, 1], F32)
            nc.vector.memset(eps_sb, EPS_EFF)

            # ---- zero-padded modulated input (bf16), border-only memsets ----
            pad_sb = []
            for j in range(NCI):
                p = pads.tile([128, HWPAD, HWPAD], BF16, tag=f"pad{j}")
                nc.gpsimd.memset(p[:, 0, :], 0.0)
                nc.gpsimd.memset(p[:, HWPAD - 1, :], 0.0)
                nc.gpsimd.memset(p[:, 1 : HWPAD - 1, 0], 0.0)
                nc.gpsimd.memset(p[:, 1 : HWPAD - 1, HWPAD - 1], 0.0)
                pad_sb.append(p)
            # serialized x chain: issue x[j+1] only after x[j] has landed
            # (1-elem read forces the wait), keeping one x transfer active
            xscr = singles.tile([1, 1], BF16)
            for j in range(NCI - 1):
                nc.gpsimd.tensor_copy(out=xscr, in_=xt_sb[j][0:1, 0, 0:1])
                nc.gpsimd.dma_start(out=xt_sb[j + 1], in_=x_ext[j + 1])
            for j in range(NCI):
                nc.vector.tensor_scalar(
                    out=pad_sb[j][:, 1 : H + 1, 1 : W + 1],
                    in0=xt_sb[j],
                    scalar1=yb_sb[:, j : j + 1],
                    scalar2=None,
                    op0=mybir.AluOpType.mult,
                )
            # ys^2 in bf16 for the demod matmuls (not needed until ~40us in)
            ys2_sb = singles.tile([128, NCI], BF16)
            nc.vector.tensor_mul(ys2_sb, yb_sb[:, 0:NCI], yb_sb[:, 0:NCI])

            xs2_ps = dps.tile([128, NCO], F32)
            rs_sb = singles.tile([128, NCO], F32)

            def conv_mms(jo, half):
                ps = cps.tile([128, 512], F32, tag="ps")
                h0 = half * 16
                idx = 0
                for j in range(NCI):
                    for k in range(KK):
                        kh, kw = divmod(k, 3)
                        rhs = pad_sb[j][:, kh + h0 : kh + h0 + 16, kw : kw + W]
                        nc.tensor.matmul(
                            out=ps,
                            lhsT=wt_slice(j, jo, k),
                            rhs=rhs,
                            start=(idx == 0),
                            stop=(idx == KK * NCI - 1),
                        )
                        idx += 1
                return ps

            def epilogue(ps, jo, half):
                ot = outs.tile([128, 512], F32, tag="ot")
                nc.scalar.activation(
                    out=ot,
                    in_=ps,
                    func=AF.Identity,
                    bias=yb_sb[:, NCI + jo : NCI + jo + 1],
                    scale=rs_sb[:, jo : jo + 1],
                )
                nc.sync.dma_start(
                    out=out_ext[jo, :, half * 512 : (half + 1) * 512], in_=ot
                )

            # NOTE: emission order IS dataflow order under Tile. The first two
            # co-tiles' matmuls are emitted before the demod chain (whose
            # q1..q3 weight quarters arrive late under the throttled stream)
            # so the PE streams as inputs land; their epilogues (which read
            # rs_sb) must come after the demod chain.
            early = [
                (conv_mms(jo, half), jo, half)
                for jo in range(2)
                for half in range(2)
            ]
            for jo in range(NCO):
                for j in range(NCI):
                    nc.tensor.matmul(
                        out=xs2_ps[:, jo : jo + 1],
                        lhsT=wt_slice(j, jo, KK),
                        rhs=ys2_sb[:, j : j + 1],
                        start=(j == 0),
                        stop=(j == NCI - 1),
                    )
            nc.scalar.activation(out=rs_sb, in_=xs2_ps, func=AF.Sqrt, bias=eps_sb)
            nc.vector.reciprocal(out=rs_sb, in_=rs_sb)
            for ps, jo, half in early:
                epilogue(ps, jo, half)
            for jo in range(2, NCO):
                for half in range(2):
                    epilogue(conv_mms(jo, half), jo, half)
            # keep the warm-up matmuls live (cheap PSUM read at the end)
            warm_sink = singles.tile([1, 1], F32)
            nc.vector.tensor_copy(out=warm_sink, in_=warm_ps[0:1, 0:1])
    nc.compile()
    return nc


_NC_CACHE = None


def _get_nc():
    global _NC_CACHE
    if _NC_CACHE is None:
        _NC_CACHE = build_nc()
    return _NC_CACHE


def _prep_inputs(x, y_s, weight, bias):
    # [co, ci, kh, kw] -> [k, ci, co]; append w2 = sum_k wt^2 as slot 9;
    # then tile to [jci, jco, ci_p, 10, co_c] bf16 contiguous.
    wt9 = weight.transpose(2, 3, 1, 0).reshape(KK, CI, CO)
    w2 = (wt9.astype(np.float64) ** 2).sum(axis=0).astype(np.float32)
    full = np.concatenate([wt9, w2[None]], axis=0)  # [10, ci, co]
    wtq = np.ascontiguousarray(
        full.reshape(KK + 1, NCI, 128, NCO, 128).transpose(1, 3, 2, 0, 4)
    ).astype(ml_dtypes.bfloat16)
    in_maps = []
    for b in range(B):
        yb = np.empty((128, 2 * NCI), np.float32)
        yb[:, :NCI] = y_s[b].reshape(NCI, 128).T
        yb[:, NCI:] = bias.reshape(NCO, 128).T
        in_maps.append(
            {
                "x": np.ascontiguousarray(x[b].reshape(NCI, 128, H, W)).astype(
                    ml_dtypes.bfloat16
                ),
                "yb": yb,
                "wt": wtq,
            }
        )
    return in_maps


def _install_trace_support():
    """Dev-only: register the axon NTFF profiling hook + disable the
    remote artifact upload so trace=True works in this container."""
    import sys
    import types

    import concourse.bass_utils as bu

    bu.upload_artifacts = lambda tmpdir: "local://" + str(tmpdir)
    if "antenv.axon_hooks" in sys.modules:
        return
    try:
        from trn_agent_boot.trn_boot import _ntff_profile_via_ctypes

        hook = _ntff_profile_via_ctypes("/opt/axon/libaxon_pjrt.so")
    except Exception:
        return
    mod = types.ModuleType("antenv.axon_hooks")
    mod.get_axon_ntff_profile_hook = lambda: hook
    mod.set_axon_ntff_profile_hook = lambda h: None
    sys.modules["antenv.axon_hooks"] = mod


def run(x, y_s, weight, bias, trace=False, tmpdir=None):
    nc = _get_nc()
    if trace:
        _install_trace_support()
    in_maps = _prep_inputs(x, y_s, weight, bias)
    res = run_bass_kernel_spmd(
        nc, in_maps, core_ids=list(range(B)), trace=trace, tmpdir=tmpdir
    )
    out = np.stack(
        [res.results[b]["out"].reshape(CO, H, W) for b in range(B)]
    ).astype(np.float32)
    return out, res


def kernel(x, y_s, weight, bias):
    out, _ = run(
        np.asarray(x, dtype=np.float32),
        np.asarray(y_s, dtype=np.float32),
        np.asarray(weight, dtype=np.float32),
        np.asarray(bias, dtype=np.float32),
    )
    return out



# revision 6
# speedup vs baseline: 1.2756x; 1.2756x over previous
"""StyleGAN2-style modulated 3x3 conv (B=8, Ci=Co=512, H=W=32) on 8 TRN2
NeuronCores, via 1-D Winograd F(2,3) along y.

Sharding: data-parallel over batch, one sample per core (embarrassingly
parallel, no collectives).

Per core the conv is decomposed with 1-D Winograd F(2,3) applied to the
ky axis: the 3 ky taps collapse into 4 transform phases a=0..3, cutting
matmul work 1.5x vs direct (192 instead of 288 N=512 matmuls):

  V_a[ty, x'] = sum_r Bt[a,r] * pad[2*ty + r, x']       (DVE, bf16)
  M_a[co]     = sum_{kx,ci} U1[a,kx,ci,co] V_a[ci][:, kx:kx+32]  (PE, PSUM fp32)
  out[2ty+0]  = (M_0 + M_1 + M_2) * rs + bias
  out[2ty+1]  = (M_1 - M_2 - M_3) * rs + bias

with Bt = [[1,0,-1,0],[0,1,1,0],[0,-1,1,0],[0,1,0,-1]] and
U1[a,kx] = sum_ky G[a,ky] w[:,:,ky,kx], G = [[1,0,0],[.5,.5,.5],
[.5,-.5,.5],[0,0,1]] — an input-independent weight transform folded on
the host (same category as the baseline's w2 fold); all input-dependent
math runs on device.

Demod is as in the baseline: conv runs on raw (unscaled) weights, and
the per-(b,co) norm uses w2 = sum_k w^2 with the eps compensated:
  out = conv / sqrt(sum_ci ys^2 * w2 + 1e-8*Ci*K^2) + bias
"""

import numpy as np
import ml_dtypes

import concourse.mybir as mybir
from concourse import bacc
from concourse.tile import TileContext
from concourse.bass_utils import run_bass_kernel_spmd

B = 8
CI = 512
CO = 512
H = W = 32
NCI = CI // 128
NCO = CO // 128
ALPHA = 4          # F(2,3): 4 transform phases
MOUT = 2           # output rows per tile
NTY = H // MOUT    # 16 y-tiles
NSLOT = ALPHA * 3 + 1  # 12 conv slots (a*3+kx) + 1 w2 slot
PADH = 34
PADW = 36          # cols: [0..1]=left border, [2..33]=x, [34..35]=right border
EPS_EFF = 1e-8 * CI * 9

F32 = mybir.dt.float32
BF16 = mybir.dt.bfloat16
AF = mybir.ActivationFunctionType
ALU = mybir.AluOpType


def build_nc():
    nc = bacc.Bacc("TRN2", target_bir_lowering=False, debug=False)

    x_ext = nc.declare_dram_parameter("x", [NCI, 128, H, W], BF16, isOutput=False)
    # cols 0..3 = y_s per ci-tile, cols 4..7 = bias per co-tile
    yb_ext = nc.declare_dram_parameter("yb", [128, 2 * NCI], F32, isOutput=False)
    # [jo, jci, ci_p, slot(12 conv + w2), co_c] bf16
    wt_ext = nc.declare_dram_parameter(
        "wt", [NCO, NCI, 128, NSLOT, 128], BF16, isOutput=False
    )
    out_ext = nc.declare_dram_parameter("out", [NCO, 128, H * W], F32, isOutput=True)

    with TileContext(nc) as tc:
        with (
            tc.tile_pool(name="singles", bufs=1) as singles,
            tc.tile_pool(name="wts", bufs=1) as wts,
            tc.tile_pool(name="pads", bufs=1) as pads,
            tc.tile_pool(name="vts", bufs=1) as vts,
            tc.tile_pool(name="xin", bufs=4) as xin,
            tc.tile_pool(name="mbs", bufs=6) as mbs,
            tc.tile_pool(name="zts", bufs=2) as zts,
            tc.tile_pool(name="outs", bufs=2) as outs,
            tc.tile_pool(name="cps", bufs=6, space="PSUM") as cps,
            tc.tile_pool(name="dps", bufs=1, space="PSUM") as dps,
            tc.tile_pool(name="wps", bufs=1, space="PSUM") as wps,
        ):
            # ---- input DMAs ----
            # x0 + yb issued from sync immediately. The x chain and the
            # weight stream are interleaved on the gpsimd queue so HBM
            # bandwidth alternates x[j] / wt[k] early on: each 1-elem
            # read forces a wait for the previous transfer to land
            # before the next DMA is issued.
            xt_sb = [
                xin.tile([128, H, W], BF16, tag=f"x{j}", name=f"xt{j}")
                for j in range(NCI)
            ]
            yb_sb = singles.tile([128, 2 * NCI], F32)
            nc.sync.dma_start(out=xt_sb[0], in_=x_ext[0])
            nc.sync.dma_start(out=yb_sb, in_=yb_ext[:, :])

            NW = NCO * NCI
            wt_sb = [None] * NW

            def wdma(k):
                w = wts.tile([128, NSLOT, 128], BF16, tag=f"wt{k}")
                nc.gpsimd.dma_start(out=w, in_=wt_ext[k // NCI, k % NCI])
                wt_sb[k] = w

            def wt_slice(jo, jci, s):
                return wt_sb[jo * NCI + jci][:, s, :]

            wscr = singles.tile([1, 1], BF16)
            xscr = singles.tile([1, 1], BF16)

            # first two weight tiles in flight alongside x0
            wdma(0)
            wdma(1)

            # pad border memsets early on gpsimd (no deps; must precede
            # the throttled weight chain below or V-transforms would
            # queue behind late weight DMAs in the gpsimd FIFO)
            pad_sb = []
            for j in range(NCI):
                p = pads.tile([128, PADH, PADW], BF16, tag=f"pad{j}")
                nc.gpsimd.memset(p[:, 0, :], 0.0)
                nc.gpsimd.memset(p[:, PADH - 1, :], 0.0)
                nc.gpsimd.memset(p[:, 1 : PADH - 1, 0:2], 0.0)
                nc.gpsimd.memset(p[:, 1 : PADH - 1, PADW - 2 : PADW], 0.0)
                pad_sb.append(p)

            # interleaved x/weight issue chain (all on gpsimd FIFO):
            # keeps <=2 weight transfers in flight while x streams, and
            # keeps the weight stream SEQUENTIAL afterwards (the DMA
            # rings round-robin across active transfers, so an
            # unthrottled burst would make the first-needed tile land
            # as late as the last one).
            for j in range(NCI - 1):
                nc.gpsimd.tensor_copy(out=xscr, in_=xt_sb[j][0:1, 0, 0:1])
                nc.gpsimd.dma_start(out=xt_sb[j + 1], in_=x_ext[j + 1])
                nc.gpsimd.tensor_copy(out=wscr, in_=wt_sb[j][0:1, 0, 0:1])
                wdma(j + 2)
            nc.gpsimd.tensor_copy(out=xscr, in_=xt_sb[NCI - 1][0:1, 0, 0:1])
            for k in range(NCI + 1, NW):
                nc.gpsimd.tensor_copy(out=wscr, in_=wt_sb[k - 2][0:1, 0, 0:1])
                wdma(k)

            # ---- PE warm-up: throwaway matmuls on memset data so the
            # HAM clock gate starts releasing before the real stream ----
            warm_lhs = singles.tile([128, 1], BF16)
            nc.vector.memset(warm_lhs, 1.0)
            warm_rhs = singles.tile([128, 512], BF16)
            nc.vector.memset(warm_rhs, 0.5)
            warm_ps = wps.tile([1, 512], F32)
            N_WARM = 8
            for i in range(N_WARM):
                nc.tensor.matmul(
                    out=warm_ps,
                    lhsT=warm_lhs,
                    rhs=warm_rhs,
                    start=(i == 0),
                    stop=(i == N_WARM - 1),
                )

            eps_sb = singles.tile([128, 1], F32)
            nc.vector.memset(eps_sb, EPS_EFF)

            # modulate + y-transform per ci-tile (DVE; 2x bf16 mode —
            # inner dim contiguous, rows 4B-aligned)
            v_sb = [[None] * NCI for _ in range(ALPHA)]
            for j in range(NCI):
                nc.vector.tensor_scalar(
                    out=pad_sb[j][:, 1 : H + 1, 2 : W + 2],
                    in0=xt_sb[j],
                    scalar1=yb_sb[:, j : j + 1],
                    scalar2=None,
                    op0=ALU.mult,
                )
                p = pad_sb[j]
                for a in range(ALPHA):
                    v = vts.tile([128, NTY, PADW], BF16, tag=f"v{a}_{j}")
                    v_sb[a][j] = v
                r = [p[:, a : a + 31 : 2, :] for a in range(ALPHA)]
                nc.vector.tensor_sub(v_sb[0][j], r[0], r[2])
                nc.vector.tensor_add(v_sb[1][j], r[1], r[2])
                nc.vector.tensor_sub(v_sb[2][j], r[2], r[1])
                nc.vector.tensor_sub(v_sb[3][j], r[1], r[3])

            # ys^2 in bf16 for the demod matmuls
            ys2_sb = singles.tile([128, NCI], BF16)
            nc.vector.tensor_mul(ys2_sb, yb_sb[:, 0:NCI], yb_sb[:, 0:NCI])

            xs2_ps = dps.tile([128, NCO], F32)
            rs_sb = singles.tile([128, NCO], F32)

            # ---- main stream: per co-tile: 4 a-phase PSUM groups of
            # 12 MMs, ACT-copy each M to SBUF bf16, demod, DVE combine,
            # ACT scale+bias into strided out rows, DMA out ----
            for jo in range(NCO):
                mb = [None] * ALPHA
                for a in range(ALPHA):
                    ps = cps.tile([128, NTY, W], F32, tag="m")
                    idx = 0
                    for jci in range(NCI):
                        for kx in range(3):
                            nc.tensor.matmul(
                                out=ps,
                                lhsT=wt_slice(jo, jci, a * 3 + kx),
                                rhs=v_sb[a][jci][:, :, kx + 1 : kx + 1 + W],
                                start=(idx == 0),
                                stop=(idx == 11),
                            )
                            idx += 1
                    m = mbs.tile([128, NTY, W], BF16, tag="mb")
                    nc.scalar.activation(out=m, in_=ps, func=AF.Copy)
                    mb[a] = m
                # demod for this co-tile (tiny MMs; weights all present
                # by now so the PE never stalls on them)
                for jci in range(NCI):
                    nc.tensor.matmul(
                        out=xs2_ps[:, jo : jo + 1],
                        lhsT=wt_slice(jo, jci, NSLOT - 1),
                        rhs=ys2_sb[:, jci : jci + 1],
                        start=(jci == 0),
                        stop=(jci == NCI - 1),
                    )
                nc.scalar.activation(
                    out=rs_sb[:, jo : jo + 1],
                    in_=xs2_ps[:, jo : jo + 1],
                    func=AF.Sqrt,
                    bias=eps_sb,
                )
                nc.vector.reciprocal(
                    out=rs_sb[:, jo : jo + 1], in_=rs_sb[:, jo : jo + 1]
                )
                # combine: Z0 = M0+M1+M2, Z1 = M1-M2-M3 (DVE bf16 2x)
                t01 = zts.tile([128, NTY, W], BF16, tag="t01")
                t23 = zts.tile([128, NTY, W], BF16, tag="t23")
                z0 = zts.tile([128, NTY, W], BF16, tag="z0")
                z1 = zts.tile([128, NTY, W], BF16, tag="z1")
                nc.vector.tensor_add(t01, mb[0], mb[1])
                nc.vector.tensor_sub(t23, mb[1], mb[2])
                nc.vector.tensor_add(z0, t01, mb[2])
                nc.vector.tensor_sub(z1, t23, mb[3])
                # epilogue: out rows 2ty+p = Z_p * rs + bias
                ot = outs.tile([128, H, W], F32, tag="ot")
                for p, z in ((0, z0), (1, z1)):
                    nc.scalar.activation(
                        out=ot[:, p : p + 31 : 2, :],
                        in_=z,
                        func=AF.Identity,
                        bias=yb_sb[:, NCI + jo : NCI + jo + 1],
                        scale=rs_sb[:, jo : jo + 1],
                    )
                nc.sync.dma_start(out=out_ext[jo], in_=ot)

            # keep the warm-up matmuls live (cheap PSUM read at the end)
            warm_sink = singles.tile([1, 1], F32)
            nc.vector.tensor_copy(out=warm_sink, in_=warm_ps[0:1, 0:1])
    nc.compile()
    return nc


_NC_CACHE = None


def _get_nc():
    global _NC_CACHE
    if _NC_CACHE is None:
        _NC_CACHE = build_nc()
    return _NC_CACHE


def _prep_inputs(x, y_s, weight, bias):
    # Winograd weight transform (input-independent): U1[a,kx,ci,co] =
    # sum_ky G[a,ky] w[co,ci,ky,kx]; slot 12 = w2 = sum_k w^2.
    G = np.array(
        [[1, 0, 0], [0.5, 0.5, 0.5], [0.5, -0.5, 0.5], [0, 0, 1]], np.float64
    )
    w64 = weight.astype(np.float64)  # [co, ci, ky, kx]
    u1 = np.einsum("ag,oigx->axio", G, w64)  # [a, kx, ci, co]
    w2 = (w64**2).sum(axis=(2, 3)).T  # [ci, co]
    full = np.concatenate(
        [u1.reshape(ALPHA * 3, CI, CO), w2[None]], axis=0
    )  # [13, ci, co]
    wtq = np.ascontiguousarray(
        full.reshape(NSLOT, NCI, 128, NCO, 128).transpose(3, 1, 2, 0, 4)
    ).astype(ml_dtypes.bfloat16)
    in_maps = []
    for b in range(B):
        yb = np.empty((128, 2 * NCI), np.float32)
        yb[:, :NCI] = y_s[b].reshape(NCI, 128).T
        yb[:, NCI:] = bias.reshape(NCO, 128).T
        in_maps.append(
            {
                "x": np.ascontiguousarray(x[b].reshape(NCI, 128, H, W)).astype(
                    ml_dtypes.bfloat16
                ),
                "yb": yb,
                "wt": wtq,
            }
        )
    return in_maps


def _install_trace_support():
    """Dev-only: register the axon NTFF profiling hook + disable the
    remote artifact upload so trace=True works in this container."""
    import sys
    import types

    import concourse.bass_utils as bu

    bu.upload_artifacts = lambda tmpdir: "local://" + str(tmpdir)
    if "antenv.axon_hooks" in sys.modules:
        return
    try:
        from trn_agent_boot.trn_boot import _ntff_profile_via_ctypes

        hook = _ntff_profile_via_ctypes("/opt/axon/libaxon_pjrt.so")
    except Exception:
        return
    mod = types.ModuleType("antenv.axon_hooks")
    mod.get_axon_ntff_profile_hook = lambda: hook
    mod.set_axon_ntff_profile_hook = lambda h: None
    sys.modules["antenv.axon_hooks"] = mod


def run(x, y_s, weight, bias, trace=False, tmpdir=None):
    nc = _get_nc()
    if trace:
        _install_trace_support()
    in_maps = _prep_inputs(x, y_s, weight, bias)
    res = run_bass_kernel_spmd(
        nc, in_maps, core_ids=list(range(B)), trace=trace, tmpdir=tmpdir
    )
    out = np.stack(
        [res.results[b]["out"].reshape(CO, H, W) for b in range(B)]
    ).astype(np.float32)
    return out, res


def kernel(x, y_s, weight, bias):
    out, _ = run(
        np.asarray(x, dtype=np.float32),
        np.asarray(y_s, dtype=np.float32),
        np.asarray(weight, dtype=np.float32),
        np.asarray(bias, dtype=np.float32),
    )
    return out
